# revision 1
# baseline (speedup 1.0000x reference)
"""CrossingNumberLoss kernel for 8 Trainium2 NeuronCores.

Math: edges -> unit direction vectors u_i in R^2; count unordered pairs
with |<u_i,u_j>| > 0.1 (normalized by E(E-1)/2).

Key transformation: for unit u, |cos dtheta| > T  <=>  cos(2 dtheta) >
c = 2T^2-1 = -0.98, with v = (cos 2theta, sin 2theta) = (x^2-y^2, 2xy) a
unit 2-vector and v_i.v_j = cos(2 dtheta).  Only ~6.4% of pairs MISS
(cos 2dphi <= -0.98), and in angle space phi = 2theta the misses for a
given edge lie in a narrow arc phi_i + pi +- beta (beta = acos(0.98) ~=
0.2 rad).  So the host sorts edges by phi.  Each unordered pair is
OWNED by exactly one endpoint, decided block-cyclically: with 128
sorted blocks of 128 edges, block b owns pairs with partner block in
(b, b+64] mod 128 (the d=64 tie goes to the block with index < 64).
A block's owned misses then live in ranks [band_bottom(b), cut(b))
where band_bottom = rank of phi_min(b)+pi-beta and cut = the +64/+65
block boundary -- a ~600-750 column window, padded backward to a fixed
L (pad pairs are crossing pairs; they cancel in examined-gt).  The
device computes only the [128, L=768] gram slab per row block (vs
[128, 8320] for full pair coverage) and counts elements > c.
misses = examined - gt exactly (each miss pair examined exactly once),
and count = C(E-z,2) - misses with z = #degenerate (zero-length) edges
whose pairs can never cross.  This is exact (not histogram-
approximate): every potential miss pair is examined on device; window
membership is pure rank geometry plus generous fp64 angle margins.

Device per 128-row block: K=6 error-compensated bf16 matmul
(hh'+lh'+hl', |err| <~ 4e-6) -> PSUM, split into a DVE piece (is_gt,
accum_out) and an ACT piece (Sign activation, accum_out) sized to
balance the 0.96 GHz DVE vs 1.2 GHz ACT count throughput.  Host does
only O(E) work: gather/normalize, double-angle, argsort, slab packing,
final scalar combine.
"""

import numpy as np
import ml_dtypes

E = 16384
NB = 128          # number of 128-edge row blocks (global)
NCORES = 8
RB = NB // NCORES  # row-blocks per core (16), contiguous assignment
T = 0.1
# crossing  <=>  v_i . v_j > C  (double-angle cosine threshold)
C = float(2.0 * np.float64(np.float32(T)) ** 2 - 1.0)
BETA = float(np.arccos(-C))
ANG_MARGIN = 1e-3  # radians of slack when building windows

_CACHE = {}


FUSE = False  # one count instruction per pair of row blocks


def _balance(L, fuse=None):
    """Split the L-wide slab into a DVE share and an ACT share with equal
    estimated count time: (120+x)/0.96 = (352+L-x)/1.2 (per-pair when
    counts are fused over two row blocks)."""
    if fuse is None:
        fuse = FUSE
    if fuse:
        x = (0.96 * (352 + 2 * L) - 144) / (2.16 * 2)
    else:
        x = (194.0 + 0.96 * L) / 2.16
    xd = int(np.clip(round(x / 16) * 16, 128, L - 112))
    return xd, L - xd


def _split_waits(nc, mybir):
    """The walrus codegen in this env caps sync waits at 1 per instruction.
    Split the excess onto same-engine NOPs (1 wait each) inserted
    immediately before the overloaded instruction."""
    cap = 1
    ctr = 0
    for f in nc.m.functions:
        for blk in f.blocks:
            insts = blk.instructions
            if not any(
                ins.sync_info is not None
                and ins.sync_info.on_wait
                and len(ins.sync_info.on_wait) > cap
                for ins in insts
            ):
                continue
            out = []
            for ins in insts:
                si = ins.sync_info
                waits = list(si.on_wait) if si is not None and si.on_wait else []
                if len(waits) > cap:
                    extra, keep = waits[:-cap], waits[-cap:]
                    for w in extra:
                        nop = mybir.InstNoOp(name=f"waitsplit_{ctr}", ins=[], outs=[])
                        ctr += 1
                        nop.engine = ins.engine
                        nop.sync_info = mybir.SyncInfo(on_wait=[w], on_update=[])
                        nc.register_instruction(nop, overwrite=True)
                        out.append(nop)
                    ins.sync_info = mybir.SyncInfo(
                        on_wait=keep,
                        on_update=list(si.on_update) if si.on_update else [],
                    )
                out.append(ins)
            blk.instructions = out


def _dedup_ldweights(nc):
    """bass lowers every matmul to Ldweights+Matmult. Within a row-block all
    matmuls share the same stationary tile, so drop consecutive Ldweights
    that reload the identical weights AP (keeping any that carry sync)."""
    removed = 0
    for f in nc.m.functions:
        for blk in f.blocks:
            insts = blk.instructions
            out = []
            last_sig = None
            for ins in insts:
                tn = type(ins).__name__
                if str(ins.engine) == "EngineType.PE":
                    if tn == "InstLdweights":
                        sig = str(ins.ins[0])
                        si = ins.sync_info
                        clean = si is None or (not si.on_wait and not si.on_update)
                        if sig == last_sig and clean:
                            removed += 1
                            continue
                        last_sig = sig
                out.append(ins)
            if removed:
                blk.instructions = out
    return removed


def _build_nc(L=768, n_loops=1, count_mode="both", staggered=False,
              dma_split=False, dummy=0, pbufs=None, fuse=False, xd=None):
    import concourse.bass as bass
    import concourse.tile as tile
    from concourse import mybir
    from contextlib import nullcontext

    f32 = mybir.dt.float32
    bf16 = mybir.dt.bfloat16
    is_gt = mybir.AluOpType.is_gt
    add = mybir.AluOpType.add
    Sign = mybir.ActivationFunctionType.Sign

    if xd is None:
        xd, xa = _balance(L, fuse)
    else:
        xa = L - xd
    assert xd <= 1024 and xa <= 1024, (L, xd, xa)
    # PSUM tiles sized to whole banks (512 f32); deeper buffering when the
    # pieces fit single banks
    wd = 512 if xd <= 512 else 1024
    wa = 512 if xa <= 512 else 1024
    if fuse:
        assert xd <= 512 and xa <= 512, (xd, xa)
        wd = wa = 512
    if pbufs is None:
        pbufs = 3 if (wd + wa) * (3 if not fuse else 6) <= 4096 - (512 if dummy else 0) else 2

    nc = bass.Bass("TRN2", target_bir_lowering=False, debug=False, num_devices=1)
    lhs = nc.dram_tensor("lhs", [6, RB * 128], bf16, kind="ExternalInput").ap()
    rhs = nc.dram_tensor("rhs", [6, RB, L], bf16, kind="ExternalInput").ap()
    acc_dve = nc.dram_tensor("acc_dve", [128, RB], f32, kind="ExternalOutput").ap()
    acc_act = nc.dram_tensor("acc_act", [128, RB], f32, kind="ExternalOutput").ap()

    with tile.TileContext(nc) as tc:
        with (
            tc.tile_pool(name="singles", bufs=1) as singles,
            tc.tile_pool(name="rpool", bufs=3) as rpool,
            tc.tile_pool(name="ppd", bufs=pbufs, space="PSUM") as ppd,
            tc.tile_pool(name="ppa", bufs=pbufs, space="PSUM") as ppa,
            tc.tile_pool(name="pw", bufs=1, space="PSUM") as pw,
        ):
            Lw = singles.tile([6, RB * 128], bf16)
            # lhs on the ACT hwdge queue so it doesn't serialize with the
            # rhs slab stream on the SP queue
            nc.scalar.dma_start(out=Lw, in_=lhs)
            bias = singles.tile([128, 1], f32)
            nc.vector.memset(bias, -C)  # sign(p - C): ACT bias = -C
            acc_d = singles.tile([128, RB], f32)
            nc.vector.memset(acc_d, 0.0)
            acc_a = singles.tile([128, RB], f32)
            nc.gpsimd.memset(acc_a, 0.0)
            # trigger the Sign ACT-table load early so its ~2.7us overlaps
            # the initial DMAs instead of stalling the first real piece
            warm = singles.tile([128, 1], f32)
            nc.scalar.activation(out=warm, in_=bias, func=Sign, bias=bias)
            # count outputs are discarded; separate per-engine trash tiles
            # keep the PSUM tiles read-only for the counters (no false
            # cross-engine write deps; tracking is tile-granular)
            tshape = [128, 2, xd] if fuse else [128, xd]
            trash_d = singles.tile(tshape, bf16)
            trash_a = singles.tile(tshape[:-1] + [xa], bf16)
            if dummy:
                # keep-warm matmul target: PE p-state drops when the engine
                # idles, and at small L the gram fills leave it ~50% idle
                pwarm = pw.tile([128, 512], f32)

            loop_cm = (
                tc.For_i(
                    0, n_loops, 1,
                    hint_engines=(
                        mybir.EngineType.PE, mybir.EngineType.DVE,
                        mybir.EngineType.Activation, mybir.EngineType.SP,
                    ),
                    staggered_reset=staggered,
                )
                if n_loops > 1
                else nullcontext()
            )

            loop_cm.__enter__()
            for pr in range(RB // 2):
                Rc = rpool.tile([6, 2, L], bf16, tag="R")
                # alternate rhs slab DMAs across the two hwdge queues
                dma_eng = nc.sync if (pr % 2 == 0 or not dma_split) else nc.scalar
                dma_eng.dma_start(out=Rc, in_=rhs[:, 2 * pr : 2 * pr + 2, :])
                if fuse:
                    Pd = ppd.tile([128, 2, wd], f32, tag="psd")
                    Pa = ppa.tile([128, 2, wa], f32, tag="psa")
                for half in range(2):
                    rb = 2 * pr + half
                    lt = Lw[:, rb * 128 : (rb + 1) * 128]
                    R = Rc[:, half, :]
                    if fuse:
                        Pdh, Pah = Pd[:, half, :], Pa[:, half, :]
                    else:
                        Pdh = ppd.tile([128, wd], f32, tag="psd")
                        Pah = ppa.tile([128, wa], f32, tag="psa")
                    for s0 in range(0, xd, 512):
                        w = min(512, xd - s0)
                        nc.tensor.matmul(
                            Pdh[:, s0 : s0 + w], lt, R[:, s0 : s0 + w],
                            start=True, stop=True,
                        )
                    for s0 in range(0, xa, 512):
                        w = min(512, xa - s0)
                        nc.tensor.matmul(
                            Pah[:, s0 : s0 + w], lt, R[:, xd + s0 : xd + s0 + w],
                            start=True, stop=True,
                        )
                    if dummy:
                        for s0 in range(0, dummy, 512):
                            w = min(512, dummy - s0)
                            nc.tensor.matmul(
                                pwarm[:, :w], lt, R[:, :w],
                                start=True, stop=True,
                            )
                    if fuse and half == 0:
                        continue
                    ind = Pd[:, :, :xd] if fuse else Pdh[:, :xd]
                    ina = Pa[:, :, :xa] if fuse else Pah[:, :xa]
                    if count_mode in ("both", "dve"):
                        nc.vector.tensor_scalar(
                            out=trash_d, in0=ind, scalar1=C,
                            scalar2=None, op0=is_gt, op1=add,
                            accum_out=acc_d[:, rb : rb + 1],
                        )
                    if count_mode in ("both", "act"):
                        nc.scalar.activation(
                            out=trash_a, in_=ina, func=Sign, bias=bias,
                            scale=1.0, accum_out=acc_a[:, rb : rb + 1],
                        )
            loop_cm.__exit__(None, None, None)
            nc.sync.dma_start(out=acc_dve, in_=acc_d)
            nc.sync.dma_start(out=acc_act, in_=acc_a)

    _dedup_ldweights(nc)
    _split_waits(nc, mybir)
    return nc


def _preprocess(node_pos, edge_index):
    """Mimic the reference's fp32 edge-vector normalization, then build the
    double-angle vectors, their sorted order, and per-block window starts."""
    node_pos = np.asarray(node_pos, dtype=np.float32)
    ei = np.asarray(edge_index).astype(np.int64)
    ev = node_pos[ei[1]] - node_pos[ei[0]]          # [E,2] f32
    nrm = np.sqrt(ev[:, 0] * ev[:, 0] + ev[:, 1] * ev[:, 1])
    u = ev / np.maximum(nrm, np.float32(1e-6))[:, None]
    # |u| <= 0.1  =>  |cos(u_i,u_j)| <= 0.1 for every partner: those edges
    # can never cross; their pairs are excluded combinatorially on host.
    z = int((nrm <= np.float32(1e-7)).sum())

    v = np.stack(
        [u[:, 0] * u[:, 0] - u[:, 1] * u[:, 1], 2.0 * u[:, 0] * u[:, 1]], axis=1
    ).astype(np.float32)                            # [E,2] double-angle
    phi = np.arctan2(v[:, 1].astype(np.float64), v[:, 0].astype(np.float64))
    order = np.argsort(phi, kind="stable")
    phis = phi[order]
    vsort = v[order]

    # One-sided cyclic windows in sorted rank space: block b owns pairs
    # with partner block in (b, b+64] mod NB (d=64 tie -> block index < 64).
    # Owned misses lie in [rank(phi_min(b)+pi-beta), cut(b)) where cut is
    # the +64/+65 block boundary; pad backward to a uniform L.
    def wrap(x):
        return np.mod(x + np.pi, 2.0 * np.pi) - np.pi

    b = np.arange(NB)
    i0 = b * 128
    lo = wrap(phis[i0] + np.pi - BETA - ANG_MARGIN)
    band_lo = np.searchsorted(phis, lo, side="left")
    endblk = np.where(b < NB // 2, b + NB // 2 + 1, b + NB // 2)
    cut = (endblk * 128) % E
    spans = (cut - band_lo) % E
    maxspan = int(spans.max())
    L = max(768, int(np.ceil((maxspan + 8) / 32.0)) * 32)
    assert L <= 2048, f"window span {maxspan} too wide for single-tile slabs"
    starts = (cut - L) % E

    hi32 = vsort.astype(ml_dtypes.bfloat16).astype(np.float32)
    hib = hi32.astype(ml_dtypes.bfloat16)
    lob = (vsort - hi32).astype(ml_dtypes.bfloat16)
    wl = np.concatenate([hib, lob, hib], axis=1).T.copy()   # [6,E] lhs rows
    wr = np.concatenate([hib, hib, lob], axis=1).T.copy()   # [6,E] rhs rows
    return z, L, starts, wl, wr


def make_in_maps(node_pos, edge_index):
    z, L, starts, wl, wr = _preprocess(node_pos, edge_index)
    wrw = np.concatenate([wr, wr[:, :L]], axis=1)   # cyclic wrap
    in_maps = []
    for c in range(NCORES):
        blks = range(c * RB, (c + 1) * RB)
        lhs = wl[:, c * RB * 128 : (c + 1) * RB * 128]
        rhs = np.stack([wrw[:, starts[b] : starts[b] + L] for b in blks], axis=1)
        in_maps.append(
            {"lhs": np.ascontiguousarray(lhs), "rhs": np.ascontiguousarray(rhs)}
        )
    meta = {"z": z, "L": L}
    return meta, in_maps


def combine(results, meta, xa=None):
    """results: list of 8 dicts with acc_dve [128,RB] and acc_act [128,RB]."""
    z, L = meta["z"], meta["L"]
    if xa is None:
        _, xa = _balance(L)
    gt = 0.0
    for r in results:
        gt += r["acc_dve"].astype(np.float64).sum()
        sigma = r["acc_act"].astype(np.float64).sum()
        gt += 0.5 * (128 * RB * xa + sigma)
    examined = float(NCORES * RB * 128 * L)
    misses = examined - gt          # each miss pair examined exactly once
    directional = 0.5 * (E - z) * (E - z - 1)
    count = directional - misses
    return np.float32(count / (E * (E - 1) / 2))


def kernel(node_pos, edge_index):
    from concourse import bass_utils

    meta, in_maps = make_in_maps(node_pos, edge_index)
    L = meta["L"]
    key = ("nc", L)
    if key not in _CACHE:
        _CACHE[key] = _build_nc(L=L)
    nc = _CACHE[key]
    try:
        res = bass_utils.run_bass_kernel_spmd(
            nc, in_maps, core_ids=list(range(NCORES))
        )
    except Exception:
        # transient device faults (NRT_EXEC_UNIT_UNRECOVERABLE) happen on
        # occasion right after a fresh process attaches; one retry suffices
        res = bass_utils.run_bass_kernel_spmd(
            nc, in_maps, core_ids=list(range(NCORES))
        )
    return combine(res.results, meta)

